# revision 30
# baseline (speedup 1.0000x reference)
"""Trainium2 Bass kernel for the quantum-control calibration loss.

Reference computation (per sample b of 2M):
    unitary[b] = prod_s exp(-i * DT*omega[b,s] * H)   (10 segments, same H)
    infid[b]   = 1 - |tr(sigma_x^H unitary[b])|^2 / 4
    loss       = mean((infedility_data[b] - infid[b])^2)

Because every step exponentiates the SAME Hamiltonian H, the factors commute
and the product collapses exactly:
    unitary[b] = exp(-i * Phi_b * H),   Phi_b = DT * sum_s omega[b,s]
With H = H0 traceless (by construction) and target = sigma_x (traceless):
    infid[b] = 1 - k*sin^2(r*Phi_b),  k = |tr(sigma_x H0)|^2 / (4 r^2)
    e_b      = d_b - infid_b = (k/2)*s_b + d'_b
    s_b      = -cos(2*r*Phi_b) = sin(two_c0*rs_b - pi/2),  rs_b = sum_s omega[b,s]
    d'_b     = d_b + (k/2 - 1)          (host-side affine, uploaded fp8)
    loss     = mean(e_b^2)

Device strategy (pure data parallel over 8 cores, 250k rows each):
  - one fp8 stream per core (2.70MB): C=7 chunks, each a [P=128, 3080B]
    block whose per-partition line is 2800B of omega (NSEG=10 x F=280,
    segment-major) followed by 280B of d'. Chunk DMAs alternate between the
    two HWDGE queues (sync/scalar) so both stream concurrently and d'
    arrives with its omega.
  - the DoubleRow identity weights are BUILT ON DEVICE (gpsimd iota with
    channel_multiplier=-1, then a DVE is_equal) - no 32KB ident DMA with
    256B descriptors gating the first LDWEIGHTS.
  - per chunk: 10-segment row-sum as 5 DoubleRow identity-matmuls into f32
    PSUM (TensorE), s = Sin(two_c0*rs - pi/2) on ScalarE (bf16), then two
    fused scalar_tensor_tensor ops: e = (k/2)*s + d' (f32) and
    acc[:,c] = sum(e*e) via accum_out.
  - host sums the 8 x 128 x C partials in f64 and divides by 2M.
"""

import math
from contextlib import ExitStack

import numpy as np

import concourse.bacc as bacc
import concourse.bass as bass
import concourse.tile as tile
from concourse import mybir
from concourse.bass_utils import run_bass_kernel_spmd

N_CORES = 8
NSEG = 10
DT = 0.1
P = 128              # SBUF partitions
C = 7                # chunks per core
F = 280              # rows per partition per chunk
OLINE = NSEG * F     # omega bytes per partition per chunk (fp8)
LINE = OLINE + F     # + d' bytes
R_PAD = C * P * F    # padded rows per core = 250_880
B_TOTAL = 2_000_000
B_LOCAL = B_TOTAL // N_CORES  # 250_000

FP8 = mybir.dt.float8e4
BF16 = mybir.dt.bfloat16
NP_FP8 = mybir.dt.np(FP8)

HAM = np.array([[0.0, 0.5], [0.5, 0.0]], dtype=np.complex64)
TARGET = np.array([[0.0, 1.0], [1.0, 0.0]], dtype=np.complex64)

_STATE: dict = {}
LAST_RESULTS = None  # BassKernelResults of the most recent device run
NEG_HALFPI = float(np.float32(-np.pi / 2))


def _drop_default_act_table_load(nc):
    """Only Sin (act-func-set 9) runs on ScalarE; the compiler also inserts a
    1.28us load of the default set 0 that nothing uses, which stalls the
    scalar sequencer right when it should be dispatching its HWDGE chunk
    DMAs. Both loads carry no sync info (verified), so dropping set 0 is
    side-effect free."""
    for func in nc.m.functions:
        for block in func.blocks:
            insts = block.instructions
            keep = [
                i
                for i in insts
                if not (
                    type(i).__name__ == "InstLoadActFuncSet"
                    and i.act_func_set_id == 0
                )
            ]
            if len(keep) != len(insts):
                block.instructions = keep


def _build_nc(two_c0: float, half_k: float) -> bass.Bass:
    nc = bacc.Bacc(None, target_bir_lowering=False, debug=False)
    f32 = mybir.dt.float32
    i32 = mybir.dt.int32
    data = nc.declare_dram_parameter("data", [C * P * LINE], FP8, isOutput=False)
    out = nc.declare_dram_parameter("partials", [P, C], f32, isOutput=True)

    with tile.TileContext(nc) as tc, ExitStack() as ctx:
        singles = ctx.enter_context(tc.tile_pool(name="singles", bufs=1))
        omp = ctx.enter_context(tc.tile_pool(name="omp", bufs=1))
        sp = ctx.enter_context(tc.tile_pool(name="sp", bufs=3))
        ep = ctx.enter_context(tc.tile_pool(name="ep", bufs=3))
        dp = ctx.enter_context(tc.tile_pool(name="dp", bufs=2))
        psump = ctx.enter_context(tc.tile_pool(name="psum", bufs=3, space="PSUM"))

        # chunk DMAs issued up-front, alternating across the two HWDGE
        # queues; every chunk owns its SBUF tile so nothing stalls on
        # buffer reuse.
        tiles = []
        for c in range(C):
            t = omp.tile([P, LINE], FP8, tag=f"ch{c}")
            eng = nc.sync if c % 2 == 0 else nc.scalar
            eng.dma_start(
                out=t,
                in_=data[c * P * LINE : (c + 1) * P * LINE].rearrange(
                    "(p x) -> p x", p=P, x=LINE
                ),
            )
            tiles.append(t)

        # identity weights built on device: iota gives (j - p) per element,
        # is_equal against 0 yields the [P, 2, P] DoubleRow identity in fp8.
        iot = singles.tile([P, 2 * P], i32)
        nc.gpsimd.iota(iot, [[0, 2], [1, P]], channel_multiplier=-1)
        ident_t = singles.tile([P, 2 * P], FP8)
        nc.vector.tensor_scalar(
            out=ident_t,
            in0=iot,
            scalar1=0.0,
            scalar2=None,
            op0=mybir.AluOpType.is_equal,
        )
        ident_v = ident_t.rearrange("p (r q) -> p r q", r=2, q=P)

        biasneg = singles.tile([P, 1], f32)
        nc.vector.memset(biasneg, NEG_HALFPI)
        acc = singles.tile([P, C], f32)

        for c in range(C):
            omv = tiles[c][:, :OLINE].rearrange("p (s f) -> p s f", s=NSEG, f=F)
            ddv = tiles[c][:, OLINE:LINE]
            # rs = sum_s omega[., s] : 5 DoubleRow identity-matmul accumulates
            rs = psump.tile([P, F], f32, tag="rs")
            for j in range(NSEG // 2):
                nc.tensor.matmul(
                    rs,
                    ident_v,
                    omv[:, 2 * j : 2 * j + 2, :],
                    start=(j == 0),
                    stop=(j == NSEG // 2 - 1),
                    perf_mode=mybir.MatmulPerfMode.DoubleRow,
                )
            # s = sin(two_c0*rs - pi/2) = -cos(2*theta)
            s_t = sp.tile([P, F], BF16, tag="s")
            nc.scalar.activation(
                out=s_t,
                in_=rs,
                func=mybir.ActivationFunctionType.Sin,
                scale=two_c0,
                bias=biasneg,
            )
            # e = (k/2)*s + d' ; acc[:, c] = sum_f e^2 (GpSimd cannot run
            # InstTensorScalarPtr through neuronxcc - probed - so both stay
            # on the DVE)
            veng = nc.vector
            e_t = ep.tile([P, F], f32, tag="e")
            veng.scalar_tensor_tensor(
                out=e_t,
                in0=s_t,
                scalar=half_k,
                in1=ddv,
                op0=mybir.AluOpType.mult,
                op1=mybir.AluOpType.add,
            )
            d2 = dp.tile([P, F], BF16, tag="d2")
            veng.scalar_tensor_tensor(
                out=d2,
                in0=e_t,
                scalar=1.0,
                in1=e_t,
                op0=mybir.AluOpType.bypass,
                op1=mybir.AluOpType.mult,
                accum_out=acc[:, c : c + 1],
            )

        nc.sync.dma_start(out=out[:, :], in_=acc)
    nc.compile()
    _drop_default_act_table_load(nc)
    return nc


def _scalar_params(x: np.ndarray):
    """Mimic the reference's f32/complex64 scalar preprocessing of the 2x2."""
    eye = np.eye(2, dtype=np.complex64)
    xc = np.asarray(x, dtype=np.float32).astype(np.complex64)
    herm = (xc + xc.T) * np.complex64(0.5) + np.complex64(1j) * (xc - xc.T) * np.complex64(0.5)
    ham_unknown = herm - np.trace(herm) * eye / np.complex64(2)
    H = HAM + ham_unknown
    tr = np.trace(H)
    H0 = H - tr * eye / np.complex64(2)
    rsq = float(np.einsum("ij,ji->", H0, H0).real) / 2.0
    r = math.sqrt(max(rsq, 1e-30))
    M = complex((TARGET.conj() * H0).sum())
    k = (abs(M) ** 2) / (4.0 * rsq) if rsq > 0 else 0.0
    return rsq, r, k


def _numpy_reference(x, omega, d):
    """Literal f32 fallback for the degenerate rsq<=1e-24 branch (never taken
    for realistic inputs; kept for exact semantic coverage)."""
    eye = np.eye(2, dtype=np.complex64)
    xc = np.asarray(x, dtype=np.float32).astype(np.complex64)
    herm = (xc + xc.T) * np.complex64(0.5) + np.complex64(1j) * (xc - xc.T) * np.complex64(0.5)
    ham_unknown = herm - np.trace(herm) * eye / np.complex64(2)
    H = HAM + ham_unknown
    tr = np.trace(H)
    H0 = H - tr * eye / np.complex64(2)
    rsq = np.float32(np.einsum("ij,ji->", H0, H0).real / 2)
    r = np.sqrt(np.maximum(rsq, np.float32(1e-30)))
    B = omega.shape[0]
    u = np.broadcast_to(eye, (B, 2, 2)).copy()
    for s in range(NSEG):
        phi = (np.float32(DT) * omega[:, s]).astype(np.float32)
        theta = phi * r
        sinc = np.where(rsq > 1e-24, np.sin(theta) / r, phi)
        phase = np.exp(np.complex64(-1j) * phi.astype(np.complex64) * tr / 2)
        u_step = phase[:, None, None] * (
            np.cos(theta).astype(np.complex64)[:, None, None] * eye
            - np.complex64(1j) * sinc.astype(np.complex64)[:, None, None] * H0
        )
        u = np.einsum("bij,bjk->bik", u_step, u)
    tmp0 = (TARGET.conj()[None] * u).sum(axis=(1, 2))
    infid = 1.0 - (tmp0 * tmp0.conj()).real / 4
    return np.float32(np.mean((d - infid) ** 2))


def kernel(para_ham_unknown, omega_data, infedility_data):
    global LAST_RESULTS
    x = np.asarray(para_ham_unknown, dtype=np.float32)
    omega = np.ascontiguousarray(np.asarray(omega_data, dtype=np.float32))
    d = np.ascontiguousarray(np.asarray(infedility_data, dtype=np.float32))

    rsq, r, k = _scalar_params(x)
    if rsq <= 1e-24:
        return _numpy_reference(x, omega, d)

    two_c0 = float(np.float32(2.0 * DT * r))
    half_k = float(np.float32(k / 2.0))
    cbias = np.float32(k / 2.0 - 1.0)

    B = omega.shape[0]
    assert B == B_TOTAL, f"kernel compiled for B={B_TOTAL}, got {B}"

    # shard + pad. row within a core = c*(P*F) + p*F + f; per-chunk device
    # block is [P, 2800B omega (segment-major) + 280B d'] per partition.
    om_pad = np.zeros((N_CORES, R_PAD, NSEG), dtype=NP_FP8)
    om_pad[:, :B_LOCAL, :] = omega.reshape(N_CORES, B_LOCAL, NSEG).astype(NP_FP8)
    # padded rows have omega=0 (s=-1) and d'=k/2 -> e = 0 contribution
    d8 = np.full((N_CORES, R_PAD), np.float32(k / 2.0), dtype=NP_FP8)
    d8[:, :B_LOCAL] = (d.reshape(N_CORES, B_LOCAL) + cbias).astype(NP_FP8)

    buf = np.empty((N_CORES, C, P, LINE), dtype=NP_FP8)
    buf[:, :, :, :OLINE] = (
        om_pad.reshape(N_CORES, C, P, F, NSEG)
        .transpose(0, 1, 2, 4, 3)
        .reshape(N_CORES, C, P, OLINE)
    )
    buf[:, :, :, OLINE:] = d8.reshape(N_CORES, C, P, F)
    buf = buf.reshape(N_CORES, C * P * LINE)

    key = (two_c0, half_k)
    if _STATE.get("key") != key:
        _STATE["nc"] = _build_nc(*key)
        _STATE["key"] = key
    nc = _STATE["nc"]

    in_maps = [{"data": buf[i]} for i in range(N_CORES)]
    res = run_bass_kernel_spmd(nc, in_maps, core_ids=list(range(N_CORES)))
    LAST_RESULTS = res

    total = 0.0
    for core_res in res.results:
        total += float(core_res["partials"].astype(np.float64).sum())
    return np.float32(total / B_TOTAL)
